# revision 31
# baseline (speedup 1.0000x reference)
"""EntityAwareBertSelfAttention Trainium2 kernel.

Data-parallel over batch: core b handles batch element b end-to-end.
Per-core math (T=1024 tokens, E=128 entities, S=1152, H=1024, 16 heads x 64):
  pos_ent = (ent + qpos) * 0.5
  q = [tok @ Wq.T ; pos_ent @ Weq.T]   (d-major layout qT [H, S])
  k = [tok @ Wk.T ; pos_ent @ Wek.T]   (d-major layout kT [H, S])
  v = [tok @ Wv.T ; ent @ Wev.T]       (seq-major layout v [S, H])
  per head h: scoresT[k,q] = kT_h.T-chunks @ qT_h   (k on partitions)
              expS = exp(scoresT/8 + mask[k])       (ScalarE, per-partition bias)
              ctx.T[d,q] & denom[q] via matmul with v augmented by a ones column
              out = ctx.T / denom
Host passes pre-transposed views (layout-only), gathers ctxT.T per core.
"""

import numpy as np
from contextlib import ExitStack

import concourse.bass as bass
import concourse.mybir as mybir
import concourse.tile as tile
from concourse import bacc
from concourse.bass_utils import run_bass_kernel_spmd

F32 = mybir.dt.float32
BF16 = mybir.dt.bfloat16

B, T, E, H, NH, HD = 8, 1024, 128, 1024, 16, 64
S = T + E  # 1152
P = 128
NCORES = 8
CI = H // P  # 8 contraction chunks
KC = S // P  # 9 key chunks
QSPLIT = [(0, 512), (512, 512), (1024, 128)]  # bank-aligned q tiles
NT = 2  # 512-wide H_out tiles for v

_CACHE = {}
# test harness hooks: set TRACE=True to profile; results stashed in LAST_RESULTS
TRACE = False
LAST_RESULTS = None


def _build(use_mask: bool, use_bias: bool):
    nc = bacc.Bacc()

    tokT = nc.declare_dram_parameter("tokT", [H, T], F32, isOutput=False)
    entT = nc.declare_dram_parameter("entT", [H, E], F32, isOutput=False)
    qposT = nc.declare_dram_parameter("qposT", [H, E], F32, isOutput=False)
    mask = nc.declare_dram_parameter("mask", [S], F32, isOutput=False)
    wt = {}
    for w in ("q", "k", "v", "eq", "ek", "ev"):
        wt[w] = nc.declare_dram_parameter(f"wt_{w}", [H, H], F32, isOutput=False)
    bias = {}
    for w in ("q", "k", "v", "eq", "ek", "ev"):
        bias[w] = nc.declare_dram_parameter(f"b_{w}", [H], F32, isOutput=False)
    ctxT = nc.declare_dram_parameter("ctxT", [H, S], F32, isOutput=True)

    with tile.TileContext(nc) as tc, ExitStack() as ctx:
        stage = ctx.enter_context(tc.tile_pool(name="stage", bufs=2))
        persist = ctx.enter_context(tc.tile_pool(name="persist", bufs=1))
        wsl = ctx.enter_context(tc.tile_pool(name="wsl", bufs=2))
        outp = ctx.enter_context(tc.tile_pool(name="outp", bufs=2))
        upool = ctx.enter_context(tc.tile_pool(name="upool", bufs=2))
        wsle = ctx.enter_context(tc.tile_pool(name="wsle", bufs=1))
        rpool = ctx.enter_context(tc.tile_pool(name="rpool", bufs=2))
        ps_acc = ctx.enter_context(tc.tile_pool(name="ps_acc", bufs=2, space="PSUM"))
        ps_s = ctx.enter_context(tc.tile_pool(name="ps_s", bufs=1, space="PSUM"))

        # ---- constants / small tensors ----
        if use_mask:
            mask_sb = persist.tile([P, KC], F32, tag="mask")
            nc.sync.dma_start(out=mask_sb[:], in_=mask.rearrange("(c p) -> p c", p=P))
        bias_sb = {}
        if use_bias:
            ones_row = persist.tile([1, P], BF16, tag="ones_row")
            nc.vector.memset(ones_row[:], 1.0)
            for w in ("q", "k", "eq", "ek"):
                t = persist.tile([P, CI], F32, tag=f"bias_{w}")
                nc.sync.dma_start(out=t[:], in_=bias[w].rearrange("(c p) -> p c", p=P))
                bias_sb[w] = t
            for w in ("v", "ev"):
                t = persist.tile([1, H], BF16, tag=f"bias_{w}")
                nc.sync.dma_start(out=t[:], in_=bias[w][None, :])
                bias_sb[w] = t

        # ---- input casts: tok.T, ent.T, pos_ent.T (bf16) ----
        xw = []  # tok.T chunks [128, T] bf16
        for c in range(CI):
            st = stage.tile([P, T], F32, tag="st_big")
            nc.sync.dma_start(out=st[:], in_=tokT[bass.ts(c, P), :])
            xb = persist.tile([P, T], BF16, tag=f"xw{c}")
            nc.scalar.copy(xb[:], st[:])
            xw.append(xb)

        # ---- persistent attention operand tensors ----
        qT = [persist.tile([P, S], BF16, tag=f"qT{co}", name=f"qT{co}") for co in range(CI)]
        kT = [persist.tile([P, S], BF16, tag=f"kT{co}", name=f"kT{co}") for co in range(CI)]
        # v with interleaved ones column: [128, 16 heads, 65] per seq chunk
        v_aug = [persist.tile([P, NH, HD + 1], BF16, tag=f"v{sc}", name=f"v{sc}") for sc in range(KC)]
        for sc in range(KC):
            nc.vector.memset(v_aug[sc][:, :, HD : HD + 1], 1.0)

        def load_wslice_cols(name, co):
            """[128, 8, 128] bf16: columns co*128..(co+1)*128 of wt[name].T,
            i.e. wt rows, chunked over contraction dim."""
            st = stage.tile([P, CI * P], F32, tag="st_big")
            for ci in range(CI):
                nc.sync.dma_start(
                    out=st[:, bass.ts(ci, P)],
                    in_=wt[name][bass.ts(ci, P), bass.ts(co, P)],
                )
            pool_ = wsle if name in ("eq", "ek") else wsl
            t = pool_.tile([P, CI, P], BF16, tag=f"wcol_{name}")
            nc.vector.tensor_copy(t[:], st[:].rearrange("p (c i) -> p c i", c=CI))
            return t

        def load_wslice_rows(wrow_pool, name, nt):
            """[128, 8, 512] bf16: columns nt*512.. of wt[name].T (rhs for v)."""
            t = wrow_pool.tile([P, CI, 512], BF16, tag="wrow")
            for cg in range(4):
                st = stage.tile([P, T], F32, tag="st_big", name="st_w2")
                nc.sync.dma_start(
                    out=st[:].rearrange("p (c i) -> p c i", c=2),
                    in_=wt[name].rearrange("(c p) o -> p c o", p=P)[
                        :, bass.ds(cg * 2, 2), bass.ds(nt * 512, 512)
                    ],
                )
                nc.scalar.copy(
                    t[:, bass.ds(cg * 2, 2), :],
                    st[:].rearrange("p (c i) -> p c i", c=2),
                )
            return t

        # ---- v projections (all up front; ctx(0) needs every chunk) ----
        def emit_v_word(wv_nt, sc, nt):
            acc = ps_acc.tile([P, 512], F32, tag="acc")
            for ci in range(CI):
                nc.tensor.matmul(
                    acc[:],
                    lhsT=xw[ci][:, bass.ts(sc, P)],
                    rhs=wv_nt[:, ci, :],
                    start=(ci == 0),
                    stop=(ci == CI - 1) if not use_bias else False,
                )
            if use_bias:
                nc.tensor.matmul(
                    acc[:],
                    lhsT=ones_row[:, :P],
                    rhs=bias_sb["v"][:, bass.ds(nt * 512, 512)],
                    start=False,
                    stop=True,
                )
            nc.scalar.copy(
                v_aug[sc][:, bass.ds(nt * 8, 8), :HD],
                acc[:].rearrange("p (h d) -> p h d", d=HD),
            )

        with tc.tile_pool(name="wrow", bufs=2) as wrow_pool:
            for nt in range(NT):
                wv_nt = load_wslice_rows(wrow_pool, "v", nt)
                for sc in range(T // P):
                    emit_v_word(wv_nt, sc, nt)
            entb, peb = [], []  # ent.T, pos_ent.T chunks [128, E] bf16
            for c in range(CI):
                se = stage.tile([P, E], F32, tag="st_e")
                nc.sync.dma_start(out=se[:], in_=entT[bass.ts(c, P), :])
                sq = stage.tile([P, E], F32, tag="st_e")
                nc.sync.dma_start(out=sq[:], in_=qposT[bass.ts(c, P), :])
                sm = stage.tile([P, E], F32, tag="st_sum")
                nc.vector.tensor_add(sm[:], se[:], sq[:])
                pe = persist.tile([P, E], BF16, tag=f"pe{c}")
                nc.vector.tensor_scalar_mul(pe[:], sm[:], 0.5)
                peb.append(pe)
                eb = persist.tile([P, E], BF16, tag=f"eb{c}")
                nc.vector.tensor_copy(eb[:], se[:])
                entb.append(eb)
            for nt in range(NT):
                wev_nt = load_wslice_rows(wrow_pool, "ev", nt)
                acc = ps_acc.tile([P, 512], F32, tag="acc")
                for ci in range(CI):
                    nc.tensor.matmul(
                        acc[:],
                        lhsT=entb[ci][:],
                        rhs=wev_nt[:, ci, :],
                        start=(ci == 0),
                        stop=(ci == CI - 1) if not use_bias else False,
                    )
                if use_bias:
                    nc.tensor.matmul(
                        acc[:],
                        lhsT=ones_row[:, :E],
                        rhs=bias_sb["ev"][:, bass.ds(nt * 512, 512)],
                        start=False,
                        stop=True,
                    )
                nc.scalar.copy(
                    v_aug[T // P][:, bass.ds(nt * 8, 8), :HD],
                    acc[:].rearrange("p (h d) -> p h d", d=HD),
                )

        exps_pool = ctx.enter_context(tc.tile_pool(name="exps", bufs=2))

        # ---- q/k projection units (emitted via filler queue) ----
        def emit_qk_word(dst, wname, co, st_i):
            """One 512-wide word-projection accumulation for qT/kT chunk co."""
            wcol = qk_wcols[(wname, co)]
            acc = ps_acc.tile([P, 512], F32, tag="acc")
            for ci in range(CI):
                nc.tensor.matmul(
                    acc[:],
                    lhsT=wcol[:, ci, :],
                    rhs=xw[ci][:, bass.ds(st_i * 512, 512)],
                    start=(ci == 0),
                    stop=(ci == CI - 1),
                )
            out_sl = dst[co][:, bass.ds(st_i * 512, 512)]
            nc.vector.tensor_copy(out_sl, acc[:])
            if use_bias:
                bname = "q" if wname == "q" else "k"
                nc.vector.tensor_tensor(
                    out_sl,
                    out_sl,
                    bias_sb[bname][:, co : co + 1].to_broadcast([P, 512]),
                    mybir.AluOpType.add,
                )

        def emit_qk_ent(dst, wname, co):
            wcol = qk_wcols[(wname, co)]
            acc = ps_acc.tile([P, 512], F32, tag="acc")
            for ci in range(CI):
                nc.tensor.matmul(
                    acc[:, :E],
                    lhsT=wcol[:, ci, :],
                    rhs=peb[ci][:],
                    start=(ci == 0),
                    stop=(ci == CI - 1),
                )
            out_sl = dst[co][:, T:S]
            nc.vector.tensor_copy(out_sl, acc[:, :E])
            if use_bias:
                bname = "eq" if wname == "eq" else "ek"
                nc.vector.tensor_tensor(
                    out_sl,
                    out_sl,
                    bias_sb[bname][:, co : co + 1].to_broadcast([P, E]),
                    mybir.AluOpType.add,
                )

        qk_wcols = {}

        def load_qk_wcols(co):
            qk_wcols[("q", co)] = load_wslice_cols("q", co)
            qk_wcols[("k", co)] = load_wslice_cols("k", co)
            qk_wcols[("eq", co)] = load_wslice_cols("eq", co)
            qk_wcols[("ek", co)] = load_wslice_cols("ek", co)

        def queue_qk_units(co):
            """Filler units projecting qT[co], kT[co] (word + entity parts)."""
            units = []
            for st_i in range(2):
                units.append(lambda st_i=st_i, co=co: emit_qk_word(qT, "q", co, st_i))
                units.append(lambda st_i=st_i, co=co: emit_qk_word(kT, "k", co, st_i))
            units.append(lambda co=co: emit_qk_ent(qT, "eq", co))
            units.append(lambda co=co: emit_qk_ent(kT, "ek", co))
            return units

        # ---- ctx units (per head: 3 q-tile MM units + one finish unit) ----
        def emit_ctx_unit(expS_t, head_idx, h, qt, us, dh):
            q0, qw = QSPLIT[qt]
            acc = ps_acc.tile([P, 512], F32, tag="acc")
            for kc in range(KC):
                nc.tensor.matmul(
                    acc[: HD + 1, :qw],
                    lhsT=v_aug[kc][:, h, :],
                    rhs=expS_t[head_idx][:, kc, bass.ds(q0, qw)],
                    start=(kc == 0),
                    stop=(kc == KC - 1),
                )
            u = upool.tile([HD + 1, 512], F32, tag=f"uev{qt}", name=f"u{qt}")
            nc.scalar.copy(u[:, :qw], acc[: HD + 1, :qw])
            nc.gpsimd.dma_start(out=dh[qt : qt + 1, :qw], in_=u[HD : HD + 1, :qw])
            us.append(u)

        def emit_ctx_finish(h, us, dh):
            rh = rpool.tile([3, 512], F32, tag="recip")
            nc.vector.reciprocal(rh[:], dh[:])
            for qt, u in enumerate(us):
                q0, qw = QSPLIT[qt]
                r1 = rpool.tile([1, 512], F32, tag="r1")
                nc.gpsimd.dma_start(out=r1[:, :qw], in_=rh[qt : qt + 1, :qw])
                rb = rpool.tile([HD, 512], F32, tag="rbcast")
                nc.gpsimd.partition_broadcast(rb[:, :qw], r1[:1, :qw])
                o = wsle.tile([HD, 512], F32, tag="osb")
                nc.vector.tensor_tensor(
                    o[:, :qw], u[:HD, :qw], rb[:, :qw], mybir.AluOpType.mult
                )
                nc.sync.dma_start(
                    out=ctxT[bass.ts(h, HD), bass.ds(q0, qw)], in_=o[:, :qw]
                )

        def queue_ctx_head(expS_t, head_idx, h):
            us = []
            dh = outp.tile([3, 512], F32, tag="dh", name=f"dh{h}")
            units = [
                (lambda qt=qt: emit_ctx_unit(expS_t, head_idx, h, qt, us, dh))
                for qt in range(3)
            ]
            units.append(lambda: emit_ctx_finish(h, us, dh))
            return units

        # ---- attention pairs with filler interleave ----
        load_qk_wcols(0)
        for u in queue_qk_units(0):  # pair 0's operands must exist before its scores
            u()
        load_qk_wcols(1)
        filler = queue_qk_units(1)

        expS_prev = None
        for p in range(CI):  # 8 head pairs; pair p uses qT[p]/kT[p]
            if p + 2 <= CI - 1:
                load_qk_wcols(p + 2)
            expS_A = exps_pool.tile([P, KC, S], BF16, tag="expSA", name="eA")
            expS_B = exps_pool.tile([P, KC, S], BF16, tag="expSB", name="eB")
            expS_t = (expS_A, expS_B)
            for kc in range(KC):
                for hi in range(2):
                    sc_t = ps_s.tile([P, 3 * 512], F32, tag=f"s{hi}")
                    rows = bass.ds(hi * HD, HD)
                    for q0, qw in QSPLIT:
                        nc.tensor.matmul(
                            sc_t[:, bass.ds(q0, qw)],
                            lhsT=kT[p][rows, bass.ts(kc, P)],
                            rhs=qT[p][rows, bass.ds(q0, qw)],
                            start=True,
                            stop=True,
                            tile_position=(hi * HD, 0),
                        )
                    nc.scalar.activation(
                        expS_t[hi][:, kc, :],
                        sc_t[:, :S],
                        mybir.ActivationFunctionType.Exp,
                        bias=(mask_sb[:, kc : kc + 1] if use_mask else 0.0),
                        scale=0.125,
                    )
                # ~2 filler units per kc slot keeps PE busy while ACT exps
                for _ in range(2):
                    if filler:
                        filler.pop(0)()
            # any unfinished q/k projections for the next pair must land now
            for u in filler:
                u()
            filler = []
            if expS_prev is not None:
                pm1 = p - 1
                qa = queue_ctx_head(expS_prev, 0, 2 * pm1)
                qb = queue_ctx_head(expS_prev, 1, 2 * pm1 + 1)
                filler += qa[:3] + qb[:3] + [qa[3], qb[3]]
            if p + 2 <= CI - 1:
                filler += queue_qk_units(p + 2)
            expS_prev = expS_t
        # drain: ctx for the last pair
        qa = queue_ctx_head(expS_prev, 0, 2 * (CI - 1))
        qb = queue_ctx_head(expS_prev, 1, 2 * (CI - 1) + 1)
        for unit in qa[:3] + qb[:3] + [qa[3], qb[3]]:
            unit()
        for u in filler:
            u()

    nc.finalize()
    return nc


def _get_nc(use_mask, use_bias):
    key = (use_mask, use_bias)
    if key not in _CACHE:
        _CACHE[key] = _build(use_mask, use_bias)
    return _CACHE[key]


def kernel(
    token_hidden_states,
    entity_hidden_states,
    attention_mask,
    query_pos,
    Wq, bq, Wk, bk, Wv, bv, Weq, beq, Wek, bek, Wev, bev,
):
    tok = np.asarray(token_hidden_states, dtype=np.float32)
    ent = np.asarray(entity_hidden_states, dtype=np.float32)
    msk = np.asarray(attention_mask, dtype=np.float32).reshape(B, S)
    qp = np.asarray(query_pos, dtype=np.float32)

    weights = {
        "wt_q": Wq, "wt_k": Wk, "wt_v": Wv,
        "wt_eq": Weq, "wt_ek": Wek, "wt_ev": Wev,
    }
    biases = {
        "b_q": bq, "b_k": bk, "b_v": bv,
        "b_eq": beq, "b_ek": bek, "b_ev": bev,
    }
    wmaps = {
        n: np.ascontiguousarray(np.asarray(a, dtype=np.float32).T)
        for n, a in weights.items()
    }
    bmaps = {n: np.ascontiguousarray(np.asarray(a, dtype=np.float32)) for n, a in biases.items()}

    use_mask = bool(np.any(msk != 0.0))
    use_bias = any(bool(np.any(b != 0.0)) for b in bmaps.values())
    nc = _get_nc(use_mask, use_bias)

    in_maps = []
    for b in range(B):
        m = {
            "tokT": np.ascontiguousarray(tok[b].T),
            "entT": np.ascontiguousarray(ent[b].T),
            "qposT": np.ascontiguousarray(qp[b].T),
            "mask": np.ascontiguousarray(msk[b]),
        }
        m.update(wmaps)
        m.update(bmaps)
        in_maps.append(m)

    global LAST_RESULTS
    kwargs = {}
    if TRACE:
        kwargs = dict(trace=True, trace_cores=[0], trace_kwargs={"title": "eabert"})
    res = run_bass_kernel_spmd(nc, in_maps, core_ids=list(range(NCORES)), **kwargs)
    LAST_RESULTS = res

    tok_ctx = np.empty((B, T, H), dtype=np.float32)
    ent_ctx = np.empty((B, E, H), dtype=np.float32)
    for b in range(B):
        full = res.results[b]["ctxT"].T  # [S, H]
        tok_ctx[b] = full[:T]
        ent_ctx[b] = full[T:]
    return tok_ctx, ent_ctx


if __name__ == "__main__":
    rng = np.random.default_rng(0)
    inp = {
        "token_hidden_states": rng.standard_normal((B, T, H), dtype=np.float32),
        "entity_hidden_states": rng.standard_normal((B, E, H), dtype=np.float32),
        "attention_mask": np.zeros((B, 1, 1, S), dtype=np.float32),
        "query_pos": rng.standard_normal((B, E, H), dtype=np.float32),
    }
    for nm, std in [("Wq", 0.02), ("Wk", 0.02), ("Wv", 0.02), ("Weq", 0.02), ("Wek", 0.02), ("Wev", 0.02)]:
        inp[nm] = (rng.standard_normal((H, H)) * std).astype(np.float32)
        inp["b" + nm[1:].lower()] = np.zeros(H, dtype=np.float32)
    out = kernel(**inp)
    print(out[0].shape, out[1].shape)


# revision 32
# speedup vs baseline: 1.0438x; 1.0438x over previous
"""EntityAwareBertSelfAttention Trainium2 kernel.

Data-parallel over batch: core b handles batch element b end-to-end.
Per-core math (T=1024 tokens, E=128 entities, S=1152, H=1024, 16 heads x 64):
  pos_ent = (ent + qpos) * 0.5
  q = [tok @ Wq.T ; pos_ent @ Weq.T]   (d-major layout qT [H, S])
  k = [tok @ Wk.T ; pos_ent @ Wek.T]   (d-major layout kT [H, S])
  v = [tok @ Wv.T ; ent @ Wev.T]       (seq-major layout v [S, H])
  per head h: scoresT[k,q] = kT_h.T-chunks @ qT_h   (k on partitions)
              expS = exp(scoresT/8 + mask[k])       (ScalarE, per-partition bias)
              ctx.T[d,q] & denom[q] via matmul with v augmented by a ones column
              out = ctx.T / denom
Host passes pre-transposed views (layout-only), gathers ctxT.T per core.
"""

import numpy as np
from contextlib import ExitStack

import concourse.bass as bass
import concourse.mybir as mybir
import concourse.tile as tile
from concourse import bacc
from concourse.bass_utils import run_bass_kernel_spmd

F32 = mybir.dt.float32
BF16 = mybir.dt.bfloat16

B, T, E, H, NH, HD = 8, 1024, 128, 1024, 16, 64
S = T + E  # 1152
P = 128
NCORES = 8
CI = H // P  # 8 contraction chunks
KC = S // P  # 9 key chunks
QSPLIT = [(0, 512), (512, 512), (1024, 128)]  # bank-aligned q tiles
NT = 2  # 512-wide H_out tiles for v

_CACHE = {}
# test harness hooks: set TRACE=True to profile; results stashed in LAST_RESULTS
TRACE = False
LAST_RESULTS = None


def _build(use_mask: bool, use_bias: bool):
    nc = bacc.Bacc()

    tokT = nc.declare_dram_parameter("tokT", [H, T], F32, isOutput=False)
    entT = nc.declare_dram_parameter("entT", [H, E], F32, isOutput=False)
    qposT = nc.declare_dram_parameter("qposT", [H, E], F32, isOutput=False)
    mask = nc.declare_dram_parameter("mask", [S], F32, isOutput=False)
    wt = {}
    for w in ("q", "k", "v", "eq", "ek", "ev"):
        wt[w] = nc.declare_dram_parameter(f"wt_{w}", [H, H], F32, isOutput=False)
    bias = {}
    for w in ("q", "k", "v", "eq", "ek", "ev"):
        bias[w] = nc.declare_dram_parameter(f"b_{w}", [H], F32, isOutput=False)
    ctxT = nc.declare_dram_parameter("ctxT", [H, S], F32, isOutput=True)

    with tile.TileContext(nc) as tc, ExitStack() as ctx:
        stage = ctx.enter_context(tc.tile_pool(name="stage", bufs=2))
        persist = ctx.enter_context(tc.tile_pool(name="persist", bufs=1))
        wsl = ctx.enter_context(tc.tile_pool(name="wsl", bufs=2))
        outp = ctx.enter_context(tc.tile_pool(name="outp", bufs=2))
        upool = ctx.enter_context(tc.tile_pool(name="upool", bufs=2))
        wsle = ctx.enter_context(tc.tile_pool(name="wsle", bufs=1))
        rpool = ctx.enter_context(tc.tile_pool(name="rpool", bufs=2))
        ps_acc = ctx.enter_context(tc.tile_pool(name="ps_acc", bufs=2, space="PSUM"))
        ps_s = ctx.enter_context(tc.tile_pool(name="ps_s", bufs=1, space="PSUM"))

        # ---- constants / small tensors ----
        if use_mask:
            mask_sb = persist.tile([P, KC], F32, tag="mask")
            nc.sync.dma_start(out=mask_sb[:], in_=mask.rearrange("(c p) -> p c", p=P))
        bias_sb = {}
        if use_bias:
            ones_row = persist.tile([1, P], BF16, tag="ones_row")
            nc.vector.memset(ones_row[:], 1.0)
            for w in ("q", "k", "eq", "ek"):
                t = persist.tile([P, CI], F32, tag=f"bias_{w}")
                nc.sync.dma_start(out=t[:], in_=bias[w].rearrange("(c p) -> p c", p=P))
                bias_sb[w] = t
            for w in ("v", "ev"):
                t = persist.tile([1, H], BF16, tag=f"bias_{w}")
                nc.sync.dma_start(out=t[:], in_=bias[w][None, :])
                bias_sb[w] = t

        # ---- input casts: tok.T, ent.T, pos_ent.T (bf16) ----
        xw = []  # tok.T chunks [128, T] bf16
        for c in range(CI):
            st = stage.tile([P, T], F32, tag="st_big")
            nc.sync.dma_start(out=st[:], in_=tokT[bass.ts(c, P), :])
            xb = persist.tile([P, T], BF16, tag=f"xw{c}")
            nc.scalar.copy(xb[:], st[:])
            xw.append(xb)

        # ---- persistent attention operand tensors ----
        qT = [persist.tile([P, S], BF16, tag=f"qT{co}", name=f"qT{co}") for co in range(CI)]
        kT = [persist.tile([P, S], BF16, tag=f"kT{co}", name=f"kT{co}") for co in range(CI)]
        # v with interleaved ones column: [128, 16 heads, 65] per seq chunk
        v_aug = [persist.tile([P, NH, HD + 1], BF16, tag=f"v{sc}", name=f"v{sc}") for sc in range(KC)]
        for sc in range(KC):
            nc.vector.memset(v_aug[sc][:, :, HD : HD + 1], 1.0)

        def load_wslice_cols(name, co):
            """[128, 8, 128] bf16: columns co*128..(co+1)*128 of wt[name].T,
            i.e. wt rows, chunked over contraction dim."""
            st = stage.tile([P, CI * P], F32, tag="st_big")
            for ci in range(CI):
                nc.sync.dma_start(
                    out=st[:, bass.ts(ci, P)],
                    in_=wt[name][bass.ts(ci, P), bass.ts(co, P)],
                )
            pool_ = wsle if name in ("eq", "ek") else wsl
            t = pool_.tile([P, CI, P], BF16, tag=f"wcol_{name}")
            nc.vector.tensor_copy(t[:], st[:].rearrange("p (c i) -> p c i", c=CI))
            return t

        def load_wslice_rows(wrow_pool, name, nt):
            """[128, 8, 512] bf16: columns nt*512.. of wt[name].T (rhs for v)."""
            t = wrow_pool.tile([P, CI, 512], BF16, tag="wrow")
            for cg in range(4):
                st = stage.tile([P, T], F32, tag="st_big", name="st_w2")
                nc.sync.dma_start(
                    out=st[:].rearrange("p (c i) -> p c i", c=2),
                    in_=wt[name].rearrange("(c p) o -> p c o", p=P)[
                        :, bass.ds(cg * 2, 2), bass.ds(nt * 512, 512)
                    ],
                )
                nc.scalar.copy(
                    t[:, bass.ds(cg * 2, 2), :],
                    st[:].rearrange("p (c i) -> p c i", c=2),
                )
            return t

        # ---- v projections (all up front; ctx(0) needs every chunk) ----
        def emit_v_word(wv_nt, sc, nt):
            acc = ps_acc.tile([P, 512], F32, tag="acc")
            for ci in range(CI):
                nc.tensor.matmul(
                    acc[:],
                    lhsT=xw[ci][:, bass.ts(sc, P)],
                    rhs=wv_nt[:, ci, :],
                    start=(ci == 0),
                    stop=(ci == CI - 1) if not use_bias else False,
                )
            if use_bias:
                nc.tensor.matmul(
                    acc[:],
                    lhsT=ones_row[:, :P],
                    rhs=bias_sb["v"][:, bass.ds(nt * 512, 512)],
                    start=False,
                    stop=True,
                )
            nc.scalar.copy(
                v_aug[sc][:, bass.ds(nt * 8, 8), :HD],
                acc[:].rearrange("p (h d) -> p h d", d=HD),
            )

        with tc.tile_pool(name="wrow", bufs=2) as wrow_pool:
            for nt in range(NT):
                wv_nt = load_wslice_rows(wrow_pool, "v", nt)
                for sc in range(T // P):
                    emit_v_word(wv_nt, sc, nt)
            entb, peb = [], []  # ent.T, pos_ent.T chunks [128, E] bf16
            for c in range(CI):
                se = stage.tile([P, E], F32, tag="st_e")
                nc.sync.dma_start(out=se[:], in_=entT[bass.ts(c, P), :])
                sq = stage.tile([P, E], F32, tag="st_e")
                nc.sync.dma_start(out=sq[:], in_=qposT[bass.ts(c, P), :])
                sm = stage.tile([P, E], F32, tag="st_sum")
                nc.vector.tensor_add(sm[:], se[:], sq[:])
                pe = persist.tile([P, E], BF16, tag=f"pe{c}")
                nc.vector.tensor_scalar_mul(pe[:], sm[:], 0.5)
                peb.append(pe)
                eb = persist.tile([P, E], BF16, tag=f"eb{c}")
                nc.vector.tensor_copy(eb[:], se[:])
                entb.append(eb)
            for nt in range(NT):
                wev_nt = load_wslice_rows(wrow_pool, "ev", nt)
                acc = ps_acc.tile([P, 512], F32, tag="acc")
                for ci in range(CI):
                    nc.tensor.matmul(
                        acc[:],
                        lhsT=entb[ci][:],
                        rhs=wev_nt[:, ci, :],
                        start=(ci == 0),
                        stop=(ci == CI - 1) if not use_bias else False,
                    )
                if use_bias:
                    nc.tensor.matmul(
                        acc[:],
                        lhsT=ones_row[:, :E],
                        rhs=bias_sb["ev"][:, bass.ds(nt * 512, 512)],
                        start=False,
                        stop=True,
                    )
                nc.scalar.copy(
                    v_aug[T // P][:, bass.ds(nt * 8, 8), :HD],
                    acc[:].rearrange("p (h d) -> p h d", d=HD),
                )

        exps_pool = ctx.enter_context(tc.tile_pool(name="exps", bufs=2))

        # ---- q/k projection units (emitted via filler queue) ----
        def emit_qk_word(dst, wname, co, st_i):
            """One 512-wide word-projection accumulation for qT/kT chunk co."""
            wcol = qk_wcols[(wname, co)]
            acc = ps_acc.tile([P, 512], F32, tag="acc")
            for ci in range(CI):
                nc.tensor.matmul(
                    acc[:],
                    lhsT=wcol[:, ci, :],
                    rhs=xw[ci][:, bass.ds(st_i * 512, 512)],
                    start=(ci == 0),
                    stop=(ci == CI - 1),
                )
            out_sl = dst[co][:, bass.ds(st_i * 512, 512)]
            nc.vector.tensor_copy(out_sl, acc[:])
            if use_bias:
                bname = "q" if wname == "q" else "k"
                nc.vector.tensor_tensor(
                    out_sl,
                    out_sl,
                    bias_sb[bname][:, co : co + 1].to_broadcast([P, 512]),
                    mybir.AluOpType.add,
                )

        def emit_qk_ent(dst, wname, co):
            wcol = qk_wcols[(wname, co)]
            acc = ps_acc.tile([P, 512], F32, tag="acc")
            for ci in range(CI):
                nc.tensor.matmul(
                    acc[:, :E],
                    lhsT=wcol[:, ci, :],
                    rhs=peb[ci][:],
                    start=(ci == 0),
                    stop=(ci == CI - 1),
                )
            out_sl = dst[co][:, T:S]
            nc.vector.tensor_copy(out_sl, acc[:, :E])
            if use_bias:
                bname = "eq" if wname == "eq" else "ek"
                nc.vector.tensor_tensor(
                    out_sl,
                    out_sl,
                    bias_sb[bname][:, co : co + 1].to_broadcast([P, E]),
                    mybir.AluOpType.add,
                )

        qk_wcols = {}

        def load_qk_wcols(co):
            qk_wcols[("q", co)] = load_wslice_cols("q", co)
            qk_wcols[("k", co)] = load_wslice_cols("k", co)
            qk_wcols[("eq", co)] = load_wslice_cols("eq", co)
            qk_wcols[("ek", co)] = load_wslice_cols("ek", co)

        def queue_qk_units(co):
            """Filler units projecting qT[co], kT[co] (word + entity parts)."""
            units = []
            for st_i in range(2):
                units.append(lambda st_i=st_i, co=co: emit_qk_word(qT, "q", co, st_i))
                units.append(lambda st_i=st_i, co=co: emit_qk_word(kT, "k", co, st_i))
            units.append(lambda co=co: emit_qk_ent(qT, "eq", co))
            units.append(lambda co=co: emit_qk_ent(kT, "ek", co))
            return units

        # ---- ctx units (per head: 3 q-tile MM units + one finish unit) ----
        def emit_ctx_unit(expS_t, head_idx, h, qt, us, dh):
            q0, qw = QSPLIT[qt]
            acc = ps_acc.tile([P, 512], F32, tag="acc")
            for kc in range(KC):
                nc.tensor.matmul(
                    acc[: HD + 1, :qw],
                    lhsT=v_aug[kc][:, h, :],
                    rhs=expS_t[head_idx][:, kc, bass.ds(q0, qw)],
                    start=(kc == 0),
                    stop=(kc == KC - 1),
                )
            u = upool.tile([HD + 1, 512], F32, tag=f"uev{qt}", name=f"u{qt}")
            nc.vector.tensor_copy(u[:, :qw], acc[: HD + 1, :qw])
            nc.gpsimd.dma_start(out=dh[qt : qt + 1, :qw], in_=u[HD : HD + 1, :qw])
            us.append(u)

        def emit_ctx_finish(h, us, dh):
            rh = rpool.tile([3, 512], F32, tag="recip")
            nc.vector.reciprocal(rh[:], dh[:])
            for qt, u in enumerate(us):
                q0, qw = QSPLIT[qt]
                r1 = rpool.tile([1, 512], F32, tag="r1")
                nc.gpsimd.dma_start(out=r1[:, :qw], in_=rh[qt : qt + 1, :qw])
                rb = rpool.tile([HD, 512], F32, tag="rbcast")
                nc.gpsimd.partition_broadcast(rb[:, :qw], r1[:1, :qw])
                o = wsle.tile([HD, 512], F32, tag="osb")
                nc.vector.tensor_tensor(
                    o[:, :qw], u[:HD, :qw], rb[:, :qw], mybir.AluOpType.mult
                )
                nc.sync.dma_start(
                    out=ctxT[bass.ts(h, HD), bass.ds(q0, qw)], in_=o[:, :qw]
                )

        def queue_ctx_head(expS_t, head_idx, h):
            us = []
            dh = outp.tile([3, 512], F32, tag="dh", name=f"dh{h}")
            units = [
                (lambda qt=qt: emit_ctx_unit(expS_t, head_idx, h, qt, us, dh))
                for qt in range(3)
            ]
            units.append(lambda: emit_ctx_finish(h, us, dh))
            return units

        # ---- attention pairs with filler interleave ----
        load_qk_wcols(0)
        for u in queue_qk_units(0):  # pair 0's operands must exist before its scores
            u()
        load_qk_wcols(1)
        filler = queue_qk_units(1)

        expS_prev = None
        for p in range(CI):  # 8 head pairs; pair p uses qT[p]/kT[p]
            if p + 2 <= CI - 1:
                load_qk_wcols(p + 2)
            expS_A = exps_pool.tile([P, KC, S], BF16, tag="expSA", name="eA")
            expS_B = exps_pool.tile([P, KC, S], BF16, tag="expSB", name="eB")
            expS_t = (expS_A, expS_B)
            for kc in range(KC):
                for hi in range(2):
                    sc_t = ps_s.tile([P, 3 * 512], F32, tag=f"s{hi}")
                    rows = bass.ds(hi * HD, HD)
                    for q0, qw in QSPLIT:
                        nc.tensor.matmul(
                            sc_t[:, bass.ds(q0, qw)],
                            lhsT=kT[p][rows, bass.ts(kc, P)],
                            rhs=qT[p][rows, bass.ds(q0, qw)],
                            start=True,
                            stop=True,
                            tile_position=(hi * HD, 0),
                        )
                    nc.scalar.activation(
                        expS_t[hi][:, kc, :],
                        sc_t[:, :S],
                        mybir.ActivationFunctionType.Exp,
                        bias=(mask_sb[:, kc : kc + 1] if use_mask else 0.0),
                        scale=0.125,
                    )
                # ~3 filler units per kc slot keeps PE busy while ACT exps
                for _ in range(3):
                    if filler:
                        filler.pop(0)()
            # any unfinished q/k projections for the next pair must land now
            for u in filler:
                u()
            filler = []
            if expS_prev is not None:
                pm1 = p - 1
                qa = queue_ctx_head(expS_prev, 0, 2 * pm1)
                qb = queue_ctx_head(expS_prev, 1, 2 * pm1 + 1)
                filler += qa[:3] + qb[:3] + [qa[3], qb[3]]
            if p + 2 <= CI - 1:
                filler += queue_qk_units(p + 2)
            expS_prev = expS_t
        # drain: ctx for the last pair
        qa = queue_ctx_head(expS_prev, 0, 2 * (CI - 1))
        qb = queue_ctx_head(expS_prev, 1, 2 * (CI - 1) + 1)
        for unit in qa[:3] + qb[:3] + [qa[3], qb[3]]:
            unit()
        for u in filler:
            u()

    nc.finalize()
    return nc


def _get_nc(use_mask, use_bias):
    key = (use_mask, use_bias)
    if key not in _CACHE:
        _CACHE[key] = _build(use_mask, use_bias)
    return _CACHE[key]


def kernel(
    token_hidden_states,
    entity_hidden_states,
    attention_mask,
    query_pos,
    Wq, bq, Wk, bk, Wv, bv, Weq, beq, Wek, bek, Wev, bev,
):
    tok = np.asarray(token_hidden_states, dtype=np.float32)
    ent = np.asarray(entity_hidden_states, dtype=np.float32)
    msk = np.asarray(attention_mask, dtype=np.float32).reshape(B, S)
    qp = np.asarray(query_pos, dtype=np.float32)

    weights = {
        "wt_q": Wq, "wt_k": Wk, "wt_v": Wv,
        "wt_eq": Weq, "wt_ek": Wek, "wt_ev": Wev,
    }
    biases = {
        "b_q": bq, "b_k": bk, "b_v": bv,
        "b_eq": beq, "b_ek": bek, "b_ev": bev,
    }
    wmaps = {
        n: np.ascontiguousarray(np.asarray(a, dtype=np.float32).T)
        for n, a in weights.items()
    }
    bmaps = {n: np.ascontiguousarray(np.asarray(a, dtype=np.float32)) for n, a in biases.items()}

    use_mask = bool(np.any(msk != 0.0))
    use_bias = any(bool(np.any(b != 0.0)) for b in bmaps.values())
    nc = _get_nc(use_mask, use_bias)

    in_maps = []
    for b in range(B):
        m = {
            "tokT": np.ascontiguousarray(tok[b].T),
            "entT": np.ascontiguousarray(ent[b].T),
            "qposT": np.ascontiguousarray(qp[b].T),
            "mask": np.ascontiguousarray(msk[b]),
        }
        m.update(wmaps)
        m.update(bmaps)
        in_maps.append(m)

    global LAST_RESULTS
    kwargs = {}
    if TRACE:
        kwargs = dict(trace=True, trace_cores=[0], trace_kwargs={"title": "eabert"})
    res = run_bass_kernel_spmd(nc, in_maps, core_ids=list(range(NCORES)), **kwargs)
    LAST_RESULTS = res

    tok_ctx = np.empty((B, T, H), dtype=np.float32)
    ent_ctx = np.empty((B, E, H), dtype=np.float32)
    for b in range(B):
        full = res.results[b]["ctxT"].T  # [S, H]
        tok_ctx[b] = full[:T]
        ent_ctx[b] = full[T:]
    return tok_ctx, ent_ctx


if __name__ == "__main__":
    rng = np.random.default_rng(0)
    inp = {
        "token_hidden_states": rng.standard_normal((B, T, H), dtype=np.float32),
        "entity_hidden_states": rng.standard_normal((B, E, H), dtype=np.float32),
        "attention_mask": np.zeros((B, 1, 1, S), dtype=np.float32),
        "query_pos": rng.standard_normal((B, E, H), dtype=np.float32),
    }
    for nm, std in [("Wq", 0.02), ("Wk", 0.02), ("Wv", 0.02), ("Weq", 0.02), ("Wek", 0.02), ("Wev", 0.02)]:
        inp[nm] = (rng.standard_normal((H, H)) * std).astype(np.float32)
        inp["b" + nm[1:].lower()] = np.zeros(H, dtype=np.float32)
    out = kernel(**inp)
    print(out[0].shape, out[1].shape)
